# revision 13
# baseline (speedup 1.0000x reference)
"""DeformConv1d Trainium2 kernel (banded-matmul formulation, v3).

Math (exact rewrite of the reference):
  s_k[t]   = clip(offsets[t,k] + k, 0, 2)
  A_kd[t]  = max(0, 1 - |s_k[t] - d|),  d in {0,1,2}   (tent weights)
  interp[(c,k), t] = sum_d A_kd[t] * x[c, t+d]
  out[o,t] = sum_{c,k} W[o,c,k] * interp[(c,k), t] + bias[o]

The interp stage runs on the TensorEngine as a banded matmul: per 126-wide
t-tile, B_k[t', t] = A_{k, t'-t}[t] is a 3-diagonal band matrix, and
interp = xT_tile^T @ B_k.  The band matrices (a cheap O(T*K) elementwise
transform of the offsets input) are prepared host-side together with the
other layout transforms and streamed in, chunked so the TensorEngine
starts within a few microseconds.  PSUM->SBUF interp copies are merged
per (tile, channel-half) and split between the Scalar and Vector engines.

Sharding: data-parallel over batch B=8 across 8 NeuronCores.
"""

import numpy as np

B, C, L = 8, 256, 4096
C_OUT = 256
T = 4094
K = 3
TW = 126        # banded tile width (stride); tiles overlap by 2 in t'
NT = 33         # ceil(4094 / 126)
MW = 512        # main-matmul N chunk (one PSUM bank)
NC8 = 8         # number of main-matmul chunks

_CACHE = {}


def _build_bass():
    import concourse.bass as bass
    import concourse.mybir as mybir
    from concourse.bass import AP
    from concourse.tile import TileContext
    from concourse.vector_clock import ScopedClock

    def _patched_drain(self, tick_clock, wait_clock):
        drain_inst = self.nc.sync.drain()
        wait_clock.add_sem_waits(
            drain_inst.ins, ScopedClock({None: tick_clock.global_clock})
        )
        si = drain_inst.ins.sync_info
        waits = list(si.on_wait) if (si and si.on_wait) else []
        if len(waits) > 1:
            drain_inst.ins.sync_info = mybir.SyncInfo(
                on_wait=[waits[0]], on_update=[]
            )
            for w in waits[1:]:
                nop = self.nc.sync.nop()
                nop.ins.sync_info = mybir.SyncInfo(on_wait=[w], on_update=[])
        self.nc.all_engine_barrier()
        popped = self.nc._tile_sem_poison_stack.pop()
        assert popped is self._sem_poison
        self.nc.clear_and_free_semaphores(list(self.sems.allocated().values()))
        self.nc.all_engine_barrier()

    TileContext._drain_and_barrier = _patched_drain

    def _split_excess_waits(nc, maxw=1):
        ctr = [0]
        for fn in nc.m.functions:
            for bb in fn.blocks:
                insts = list(bb.instructions)
                out, changed = [], False
                for inst in insts:
                    si = inst.sync_info
                    waits = list(si.on_wait) if (si and si.on_wait) else []
                    if len(waits) > maxw:
                        for w in waits[:-maxw]:
                            nop = mybir.InstNoOp(
                                name=f"I-wsplit{ctr[0]}", ins=[], outs=[]
                            )
                            ctr[0] += 1
                            nop.engine = inst.engine
                            nop.sync_info = mybir.SyncInfo(
                                on_wait=[w], on_update=[]
                            )
                            out.append(nop)
                        inst.sync_info = mybir.SyncInfo(
                            on_wait=waits[-maxw:],
                            on_update=list(si.on_update) if si.on_update else [],
                        )
                        changed = True
                    out.append(inst)
                if changed:
                    bb.instructions = out

    fp32 = mybir.dt.float32
    bf16 = mybir.dt.bfloat16
    Op = mybir.AluOpType

    nc = bass.Bass()
    xtp_d = nc.dram_tensor("xtp", [128, NT * 256], bf16, kind="ExternalInput")
    band_d = nc.dram_tensor("band", [128, NT * 3 * TW], bf16,
                            kind="ExternalInput")
    wt_d = nc.dram_tensor("wt", [128, 12 * 128], bf16, kind="ExternalInput")
    bias_d = nc.dram_tensor("bias", [128, 2], fp32, kind="ExternalInput")
    out_d = nc.dram_tensor("out", [C_OUT, T], fp32, kind="ExternalOutput")

    lg = [(0, 6), (6, 14), (14, 22), (22, 28), (28, NT)]
    # main-matmul chunk groups, interleaved into the interp stream once the
    # tiles covering them are copied: group g covers chunks cgs[g].
    # 2-chunk groups + psum_o bufs=4 let oh=1 matmuls proceed while oh=0
    # PSUM banks drain.
    cgs = [[0, 1], [2, 3], [4, 5], [6, 7]]

    with TileContext(nc) as tc:
        with (
            tc.tile_pool(name="persist", bufs=1) as persist,
            tc.tile_pool(name="ipool", bufs=1) as ipool,
            tc.tile_pool(name="small", bufs=1) as small,
            tc.tile_pool(name="work", bufs=2) as work,
            tc.tile_pool(name="psum_i", bufs=4, space="PSUM") as psum_i,
            tc.tile_pool(name="psum_o", bufs=4, space="PSUM") as psum_o,
        ):
            # ---- input loads, chunked + dual-queue so PE starts early ----
            xsb = persist.tile([128, NT * 256], bf16, name="xsb", tag="xsb")
            band = persist.tile([128, NT, 3, TW], bf16, name="band", tag="band")
            wsb = persist.tile([128, 12 * 128], bf16, name="wsb", tag="wsb")
            bsb = small.tile([128, 2], fp32, name="bsb", tag="bsb")

            def load_chunk(j0, j1):
                nc.sync.dma_start(
                    xsb[:, 256 * j0 : 256 * j1], xtp_d[:, 256 * j0 : 256 * j1]
                )
                nc.scalar.dma_start(
                    band[:, j0:j1, :, :],
                    band_d[:, 378 * j0 : 378 * j1].rearrange(
                        "p (j k t) -> p j k t", k=3, t=TW
                    ),
                )

            load_chunk(*lg[0])
            nc.scalar.dma_start(wsb[:, :], wt_d[:, :])
            nc.sync.dma_start(bsb[:, :], bias_d[:, :])
            for (j0, j1) in lg[1:]:
                load_chunk(j0, j1)

            # interp2[ch][p, k, 126 j + t]
            interp2 = [
                ipool.tile([128, 3, NT * TW], bf16, name=f"it{ch}", tag=f"it{ch}")
                for ch in range(2)
            ]

            def interp_tile(j):
                for ch in range(2):
                    pi = psum_i.tile([128, 3 * TW], fp32, name="pi", tag="pi")
                    nc.tensor.matmul(
                        pi[:, :],
                        xsb[:, 256 * j + 128 * ch : 256 * j + 128 * (ch + 1)],
                        band[:, j, :, :],
                        start=True,
                        stop=True,
                    )
                    dst = AP(interp2[ch].tensor, j * TW,
                             [(3 * NT * TW, 128), (NT * TW, 3), (1, TW)])
                    src = pi[:, :].rearrange("p (k t) -> p k t", k=3)
                    if (j * 2 + ch) % 2 == 0:
                        nc.scalar.copy(dst, src)
                    else:
                        nc.vector.tensor_scalar(dst, src, 0.0, None, Op.add)

            def main_group(chunks):
                for oh in range(2):
                    pos, ws = [], []
                    for c8 in chunks:
                        n0 = MW * c8
                        ws.append(min(MW, T - n0))
                        pos.append(
                            psum_o.tile([128, MW], fp32, name="po", tag="po")
                        )
                    for bi, (k, ch) in enumerate(
                        [(k, ch) for k in range(K) for ch in range(2)]
                    ):
                        idx = (k * 2 + ch) * 2 + oh
                        for ci, c8 in enumerate(chunks):
                            n0 = MW * c8
                            w = ws[ci]
                            nc.tensor.matmul(
                                pos[ci][:, :w],
                                wsb[:, 128 * idx : 128 * (idx + 1)],
                                interp2[ch][:, k, n0 : n0 + w],
                                start=(bi == 0),
                                stop=(bi == 5),
                            )
                    for ci, c8 in enumerate(chunks):
                        n0 = MW * c8
                        w = ws[ci]
                        ost = work.tile([128, MW], fp32, name="ost", tag="ost")
                        if ci % 2 == 1:
                            nc.vector.tensor_scalar(
                                ost[:, :w], pos[ci][:, :w],
                                bsb[:, oh : oh + 1], None, Op.add,
                            )
                        else:
                            nc.scalar.activation(
                                ost[:, :w], pos[ci][:, :w],
                                mybir.ActivationFunctionType.Identity,
                                bias=bsb[:, oh : oh + 1], scale=1.0,
                            )
                        nc.sync.dma_start(
                            out_d[128 * oh : 128 * (oh + 1), n0 : n0 + w],
                            ost[:, :w],
                        )

            # interleave: emit each main group right after the interp tiles
            # covering it, so PE fills copy-bound gaps with main matmuls
            next_group = 0
            for j in range(NT):
                interp_tile(j)
                if next_group < len(cgs):
                    last_chunk = cgs[next_group][-1]
                    need_j = min(
                        NT - 1, (MW * (last_chunk + 1) + TW - 1) // TW - 1
                    )
                    if j >= need_j:
                        main_group(cgs[next_group])
                        next_group += 1
            while next_group < len(cgs):
                main_group(cgs[next_group])
                next_group += 1
    _split_excess_waits(nc)
    return nc


def _prep_inputs(x, offsets, weight, bias):
    import ml_dtypes

    bf = ml_dtypes.bfloat16
    # xtp[p, j, c] = x[c, 126 j + p]  (zero beyond L)
    jj, pp = np.meshgrid(np.arange(NT), np.arange(128), indexing="ij")
    tt = TW * jj + pp  # [NT, 128]
    valid = tt < L
    tt_c = np.clip(tt, 0, L - 1)
    xtp_all = np.zeros((B, 128, NT, 256), np.float32)
    for b in range(B):
        g = x[b][:, tt_c]  # [256, NT, 128]
        g = np.where(valid[None, :, :], g, 0.0)
        xtp_all[b] = g.transpose(2, 1, 0)  # [128, NT, 256]
    xtp = xtp_all.reshape(B, 128, NT * 256).astype(bf)

    # band[p, j, k, t] = A_{k, p-t}[126 j + t]  (tent weights on diagonals)
    # s_k[t] = clip(offsets[t, k] + k, 0, 2); A_kd[t] = max(0, 1 - |s_k - d|)
    kk = np.arange(K, dtype=np.float32)
    dd = np.arange(3, dtype=np.float32)
    band_all = np.zeros((B, 128, NT, K, TW), np.float32)
    t_idx = np.arange(TW)
    for b in range(B):
        s = np.clip(offsets[b, 0] + kk[None, :], 0.0, 2.0)  # [T, K]
        A = np.maximum(0.0, 1.0 - np.abs(s[:, :, None] - dd[None, None, :]))
        A_pad = np.zeros((NT * TW, K, 3), np.float32)
        A_pad[:T] = A  # zero beyond T: padded columns produce zero interp
        A_r = A_pad.reshape(NT, TW, K, 3)
        for d in range(3):
            # rows p = t + d, cols t  (both sides index-advance to [126,NT,K])
            band_all[b, t_idx + d, :, :, t_idx] = A_r[:, t_idx, :, d]
    band_h = band_all.reshape(B, 128, NT * K * TW).astype(bf)

    wtt = np.zeros((128, K, 2, 2, 128), np.float32)
    for k in range(K):
        for ch in range(2):
            for oh in range(2):
                wtt[:, k, ch, oh, :] = weight[
                    128 * oh : 128 * (oh + 1), 128 * ch : 128 * (ch + 1), k
                ].T
    wt = wtt.reshape(128, 12 * 128).astype(bf)

    bias2 = bias.reshape(2, 128).T.astype(np.float32).copy()

    maps = []
    for b in range(B):
        maps.append(
            {
                "xtp": np.ascontiguousarray(xtp[b]),
                "band": np.ascontiguousarray(band_h[b]),
                "wt": wt,
                "bias": bias2,
            }
        )
    return maps


def kernel(x, offsets, weight, bias):
    from concourse import bass_utils

    x = np.asarray(x, np.float32)
    offsets = np.asarray(offsets, np.float32)
    weight = np.asarray(weight, np.float32)
    bias = np.asarray(bias, np.float32)

    if "nc" not in _CACHE:
        _CACHE["nc"] = _build_bass()
    nc = _CACHE["nc"]
    in_maps = _prep_inputs(x, offsets, weight, bias)
    res = bass_utils.run_bass_kernel_spmd(nc, in_maps, core_ids=list(range(B)))
    out = np.stack([res.results[b]["out"] for b in range(B)], axis=0)
    return out.astype(np.float32)


# revision 15
# speedup vs baseline: 1.0099x; 1.0099x over previous
"""DeformConv1d Trainium2 kernel (banded-matmul formulation, v3).

Math (exact rewrite of the reference):
  s_k[t]   = clip(offsets[t,k] + k, 0, 2)
  A_kd[t]  = max(0, 1 - |s_k[t] - d|),  d in {0,1,2}   (tent weights)
  interp[(c,k), t] = sum_d A_kd[t] * x[c, t+d]
  out[o,t] = sum_{c,k} W[o,c,k] * interp[(c,k), t] + bias[o]

The interp stage runs on the TensorEngine as a banded matmul: per 126-wide
t-tile, B_k[t', t] = A_{k, t'-t}[t] is a 3-diagonal band matrix, and
interp = xT_tile^T @ B_k.  The band matrices (a cheap O(T*K) elementwise
transform of the offsets input) are prepared host-side together with the
other layout transforms and streamed in, chunked so the TensorEngine
starts within a few microseconds.  PSUM->SBUF interp copies are merged
per (tile, channel-half) and split between the Scalar and Vector engines.

Sharding: data-parallel over batch B=8 across 8 NeuronCores.
"""

import numpy as np

B, C, L = 8, 256, 4096
C_OUT = 256
T = 4094
K = 3
TW = 126        # banded tile width (stride); tiles overlap by 2 in t'
NT = 33         # ceil(4094 / 126)
MW = 512        # main-matmul N chunk (one PSUM bank)
NC8 = 8         # number of main-matmul chunks

_CACHE = {}


def _build_bass():
    import concourse.bass as bass
    import concourse.mybir as mybir
    from concourse.bass import AP
    from concourse.tile import TileContext
    from concourse.vector_clock import ScopedClock

    def _patched_drain(self, tick_clock, wait_clock):
        drain_inst = self.nc.sync.drain()
        wait_clock.add_sem_waits(
            drain_inst.ins, ScopedClock({None: tick_clock.global_clock})
        )
        si = drain_inst.ins.sync_info
        waits = list(si.on_wait) if (si and si.on_wait) else []
        if len(waits) > 1:
            drain_inst.ins.sync_info = mybir.SyncInfo(
                on_wait=[waits[0]], on_update=[]
            )
            for w in waits[1:]:
                nop = self.nc.sync.nop()
                nop.ins.sync_info = mybir.SyncInfo(on_wait=[w], on_update=[])
        self.nc.all_engine_barrier()
        popped = self.nc._tile_sem_poison_stack.pop()
        assert popped is self._sem_poison
        self.nc.clear_and_free_semaphores(list(self.sems.allocated().values()))
        self.nc.all_engine_barrier()

    TileContext._drain_and_barrier = _patched_drain

    def _split_excess_waits(nc, maxw=1):
        ctr = [0]
        for fn in nc.m.functions:
            for bb in fn.blocks:
                insts = list(bb.instructions)
                out, changed = [], False
                for inst in insts:
                    si = inst.sync_info
                    waits = list(si.on_wait) if (si and si.on_wait) else []
                    if len(waits) > maxw:
                        for w in waits[:-maxw]:
                            nop = mybir.InstNoOp(
                                name=f"I-wsplit{ctr[0]}", ins=[], outs=[]
                            )
                            ctr[0] += 1
                            nop.engine = inst.engine
                            nop.sync_info = mybir.SyncInfo(
                                on_wait=[w], on_update=[]
                            )
                            out.append(nop)
                        inst.sync_info = mybir.SyncInfo(
                            on_wait=waits[-maxw:],
                            on_update=list(si.on_update) if si.on_update else [],
                        )
                        changed = True
                    out.append(inst)
                if changed:
                    bb.instructions = out

    fp32 = mybir.dt.float32
    bf16 = mybir.dt.bfloat16
    Op = mybir.AluOpType

    nc = bass.Bass()
    xtp_d = nc.dram_tensor("xtp", [128, NT * 256], bf16, kind="ExternalInput")
    band_d = nc.dram_tensor("band", [128, NT * 3 * TW], bf16,
                            kind="ExternalInput")
    wt_d = nc.dram_tensor("wt", [128, 12 * 128], bf16, kind="ExternalInput")
    bias_d = nc.dram_tensor("bias", [128, 2], fp32, kind="ExternalInput")
    out_d = nc.dram_tensor("out", [C_OUT, T], fp32, kind="ExternalOutput")

    lg = [(0, 6), (6, 14), (14, 22), (22, 28), (28, NT)]
    # main-matmul chunk groups, interleaved into the interp stream once the
    # tiles covering them are copied: group g covers chunks cgs[g]
    cgs = [[0, 1, 2], [3, 4, 5], [6, 7]]

    with TileContext(nc) as tc:
        with (
            tc.tile_pool(name="persist", bufs=1) as persist,
            tc.tile_pool(name="ipool", bufs=1) as ipool,
            tc.tile_pool(name="small", bufs=1) as small,
            tc.tile_pool(name="work", bufs=2) as work,
            tc.tile_pool(name="psum_i", bufs=5, space="PSUM") as psum_i,
            tc.tile_pool(name="psum_o", bufs=3, space="PSUM") as psum_o,
        ):
            # ---- input loads, chunked + dual-queue so PE starts early ----
            xsb = persist.tile([128, NT * 256], bf16, name="xsb", tag="xsb")
            band = persist.tile([128, NT, 3, TW], bf16, name="band", tag="band")
            wsb = persist.tile([128, 12 * 128], bf16, name="wsb", tag="wsb")
            bsb = small.tile([128, 2], fp32, name="bsb", tag="bsb")

            def load_chunk(j0, j1):
                nc.sync.dma_start(
                    xsb[:, 256 * j0 : 256 * j1], xtp_d[:, 256 * j0 : 256 * j1]
                )
                nc.scalar.dma_start(
                    band[:, j0:j1, :, :],
                    band_d[:, 378 * j0 : 378 * j1].rearrange(
                        "p (j k t) -> p j k t", k=3, t=TW
                    ),
                )

            load_chunk(*lg[0])
            nc.scalar.dma_start(wsb[:, :], wt_d[:, :])
            nc.sync.dma_start(bsb[:, :], bias_d[:, :])
            for (j0, j1) in lg[1:]:
                load_chunk(j0, j1)

            # interp2[ch][p, k, 126 j + t]
            interp2 = [
                ipool.tile([128, 3, NT * TW], bf16, name=f"it{ch}", tag=f"it{ch}")
                for ch in range(2)
            ]

            def interp_tile(j):
                for ch in range(2):
                    pi = psum_i.tile([128, 3 * TW], fp32, name="pi", tag="pi")
                    nc.tensor.matmul(
                        pi[:, :],
                        xsb[:, 256 * j + 128 * ch : 256 * j + 128 * (ch + 1)],
                        band[:, j, :, :],
                        start=True,
                        stop=True,
                    )
                    dst = AP(interp2[ch].tensor, j * TW,
                             [(3 * NT * TW, 128), (NT * TW, 3), (1, TW)])
                    src = pi[:, :].rearrange("p (k t) -> p k t", k=3)
                    if (j * 2 + ch) % 2 == 0:
                        nc.scalar.copy(dst, src)
                    else:
                        nc.vector.tensor_scalar(dst, src, 0.0, None, Op.add)

            def main_group(chunks):
                for oh in range(2):
                    pos, ws = [], []
                    for c8 in chunks:
                        n0 = MW * c8
                        ws.append(min(MW, T - n0))
                        pos.append(
                            psum_o.tile([128, MW], fp32, name="po", tag="po")
                        )
                    for bi, (k, ch) in enumerate(
                        [(k, ch) for k in range(K) for ch in range(2)]
                    ):
                        idx = (k * 2 + ch) * 2 + oh
                        for ci, c8 in enumerate(chunks):
                            n0 = MW * c8
                            w = ws[ci]
                            nc.tensor.matmul(
                                pos[ci][:, :w],
                                wsb[:, 128 * idx : 128 * (idx + 1)],
                                interp2[ch][:, k, n0 : n0 + w],
                                start=(bi == 0),
                                stop=(bi == 5),
                            )
                    for ci, c8 in enumerate(chunks):
                        n0 = MW * c8
                        w = ws[ci]
                        ost = work.tile([128, MW], fp32, name="ost", tag="ost")
                        if ci % 2 == 1:
                            nc.vector.tensor_scalar(
                                ost[:, :w], pos[ci][:, :w],
                                bsb[:, oh : oh + 1], None, Op.add,
                            )
                        else:
                            nc.scalar.activation(
                                ost[:, :w], pos[ci][:, :w],
                                mybir.ActivationFunctionType.Identity,
                                bias=bsb[:, oh : oh + 1], scale=1.0,
                            )
                        nc.sync.dma_start(
                            out_d[128 * oh : 128 * (oh + 1), n0 : n0 + w],
                            ost[:, :w],
                        )

            # interleave: emit each main group right after the interp tiles
            # covering it, so PE fills copy-bound gaps with main matmuls
            next_group = 0
            for j in range(NT):
                interp_tile(j)
                if next_group < len(cgs):
                    last_chunk = cgs[next_group][-1]
                    need_j = min(
                        NT - 1, (MW * (last_chunk + 1) + TW - 1) // TW - 1
                    )
                    if j >= need_j:
                        main_group(cgs[next_group])
                        next_group += 1
            while next_group < len(cgs):
                main_group(cgs[next_group])
                next_group += 1
    _split_excess_waits(nc)
    return nc


def _prep_inputs(x, offsets, weight, bias):
    import ml_dtypes

    bf = ml_dtypes.bfloat16
    # xtp[p, j, c] = x[c, 126 j + p]  (zero beyond L)
    jj, pp = np.meshgrid(np.arange(NT), np.arange(128), indexing="ij")
    tt = TW * jj + pp  # [NT, 128]
    valid = tt < L
    tt_c = np.clip(tt, 0, L - 1)
    xtp_all = np.zeros((B, 128, NT, 256), np.float32)
    for b in range(B):
        g = x[b][:, tt_c]  # [256, NT, 128]
        g = np.where(valid[None, :, :], g, 0.0)
        xtp_all[b] = g.transpose(2, 1, 0)  # [128, NT, 256]
    xtp = xtp_all.reshape(B, 128, NT * 256).astype(bf)

    # band[p, j, k, t] = A_{k, p-t}[126 j + t]  (tent weights on diagonals)
    # s_k[t] = clip(offsets[t, k] + k, 0, 2); A_kd[t] = max(0, 1 - |s_k - d|)
    kk = np.arange(K, dtype=np.float32)
    dd = np.arange(3, dtype=np.float32)
    band_all = np.zeros((B, 128, NT, K, TW), np.float32)
    t_idx = np.arange(TW)
    for b in range(B):
        s = np.clip(offsets[b, 0] + kk[None, :], 0.0, 2.0)  # [T, K]
        A = np.maximum(0.0, 1.0 - np.abs(s[:, :, None] - dd[None, None, :]))
        A_pad = np.zeros((NT * TW, K, 3), np.float32)
        A_pad[:T] = A  # zero beyond T: padded columns produce zero interp
        A_r = A_pad.reshape(NT, TW, K, 3)
        for d in range(3):
            # rows p = t + d, cols t  (both sides index-advance to [126,NT,K])
            band_all[b, t_idx + d, :, :, t_idx] = A_r[:, t_idx, :, d]
    band_h = band_all.reshape(B, 128, NT * K * TW).astype(bf)

    wtt = np.zeros((128, K, 2, 2, 128), np.float32)
    for k in range(K):
        for ch in range(2):
            for oh in range(2):
                wtt[:, k, ch, oh, :] = weight[
                    128 * oh : 128 * (oh + 1), 128 * ch : 128 * (ch + 1), k
                ].T
    wt = wtt.reshape(128, 12 * 128).astype(bf)

    bias2 = bias.reshape(2, 128).T.astype(np.float32).copy()

    maps = []
    for b in range(B):
        maps.append(
            {
                "xtp": np.ascontiguousarray(xtp[b]),
                "band": np.ascontiguousarray(band_h[b]),
                "wt": wt,
                "bias": bias2,
            }
        )
    return maps


def kernel(x, offsets, weight, bias):
    from concourse import bass_utils

    x = np.asarray(x, np.float32)
    offsets = np.asarray(offsets, np.float32)
    weight = np.asarray(weight, np.float32)
    bias = np.asarray(bias, np.float32)

    if "nc" not in _CACHE:
        _CACHE["nc"] = _build_bass()
    nc = _CACHE["nc"]
    in_maps = _prep_inputs(x, offsets, weight, bias)
    res = bass_utils.run_bass_kernel_spmd(nc, in_maps, core_ids=list(range(B)))
    out = np.stack([res.results[b]["out"] for b in range(B)], axis=0)
    return out.astype(np.float32)


# revision 16
# speedup vs baseline: 1.0810x; 1.0703x over previous
"""DeformConv1d Trainium2 kernel (banded-matmul formulation, v3).

Math (exact rewrite of the reference):
  s_k[t]   = clip(offsets[t,k] + k, 0, 2)
  A_kd[t]  = max(0, 1 - |s_k[t] - d|),  d in {0,1,2}   (tent weights)
  interp[(c,k), t] = sum_d A_kd[t] * x[c, t+d]
  out[o,t] = sum_{c,k} W[o,c,k] * interp[(c,k), t] + bias[o]

The interp stage runs on the TensorEngine as a banded matmul: per 126-wide
t-tile, B_k[t', t] = A_{k, t'-t}[t] is a 3-diagonal band matrix, and
interp = xT_tile^T @ B_k.  The band matrices (a cheap O(T*K) elementwise
transform of the offsets input) are prepared host-side together with the
other layout transforms and streamed in, chunked so the TensorEngine
starts within a few microseconds.  PSUM->SBUF interp copies are merged
per (tile, channel-half) and split between the Scalar and Vector engines.

Sharding: data-parallel over batch B=8 across 8 NeuronCores.
"""

import numpy as np

B, C, L = 8, 256, 4096
C_OUT = 256
T = 4094
K = 3
TW = 126        # banded tile width (stride); tiles overlap by 2 in t'
NT = 33         # ceil(4094 / 126)
MW = 512        # main-matmul N chunk (one PSUM bank)
NC8 = 8         # number of main-matmul chunks

_CACHE = {}


def _build_bass():
    import concourse.bass as bass
    import concourse.mybir as mybir
    from concourse.bass import AP
    from concourse.tile import TileContext
    from concourse.vector_clock import ScopedClock

    def _patched_drain(self, tick_clock, wait_clock):
        drain_inst = self.nc.sync.drain()
        wait_clock.add_sem_waits(
            drain_inst.ins, ScopedClock({None: tick_clock.global_clock})
        )
        si = drain_inst.ins.sync_info
        waits = list(si.on_wait) if (si and si.on_wait) else []
        if len(waits) > 1:
            drain_inst.ins.sync_info = mybir.SyncInfo(
                on_wait=[waits[0]], on_update=[]
            )
            for w in waits[1:]:
                nop = self.nc.sync.nop()
                nop.ins.sync_info = mybir.SyncInfo(on_wait=[w], on_update=[])
        self.nc.all_engine_barrier()
        popped = self.nc._tile_sem_poison_stack.pop()
        assert popped is self._sem_poison
        self.nc.clear_and_free_semaphores(list(self.sems.allocated().values()))
        self.nc.all_engine_barrier()

    TileContext._drain_and_barrier = _patched_drain

    def _split_excess_waits(nc, maxw=1):
        ctr = [0]
        for fn in nc.m.functions:
            for bb in fn.blocks:
                insts = list(bb.instructions)
                out, changed = [], False
                for inst in insts:
                    si = inst.sync_info
                    waits = list(si.on_wait) if (si and si.on_wait) else []
                    if len(waits) > maxw:
                        for w in waits[:-maxw]:
                            nop = mybir.InstNoOp(
                                name=f"I-wsplit{ctr[0]}", ins=[], outs=[]
                            )
                            ctr[0] += 1
                            nop.engine = inst.engine
                            nop.sync_info = mybir.SyncInfo(
                                on_wait=[w], on_update=[]
                            )
                            out.append(nop)
                        inst.sync_info = mybir.SyncInfo(
                            on_wait=waits[-maxw:],
                            on_update=list(si.on_update) if si.on_update else [],
                        )
                        changed = True
                    out.append(inst)
                if changed:
                    bb.instructions = out

    fp32 = mybir.dt.float32
    bf16 = mybir.dt.bfloat16
    Op = mybir.AluOpType

    nc = bass.Bass()
    xtp_d = nc.dram_tensor("xtp", [128, NT * 256], bf16, kind="ExternalInput")
    band_d = nc.dram_tensor("band", [128, NT * 3 * TW], bf16,
                            kind="ExternalInput")
    wt_d = nc.dram_tensor("wt", [128, 12 * 128], bf16, kind="ExternalInput")
    bias_d = nc.dram_tensor("bias", [128, 2], fp32, kind="ExternalInput")
    out_d = nc.dram_tensor("out", [C_OUT, T], fp32, kind="ExternalOutput")

    lg = [(0, 4), (4, 12), (12, 20), (20, 27), (27, NT)]
    # main-matmul chunk groups, interleaved into the interp stream once the
    # tiles covering them are copied: group g covers chunks cgs[g]
    cgs = [[0, 1, 2], [3, 4, 5], [6, 7]]

    with TileContext(nc) as tc:
        with (
            tc.tile_pool(name="persist", bufs=1) as persist,
            tc.tile_pool(name="ipool", bufs=1) as ipool,
            tc.tile_pool(name="small", bufs=1) as small,
            tc.tile_pool(name="work", bufs=2) as work,
            tc.tile_pool(name="psum_i", bufs=5, space="PSUM") as psum_i,
            tc.tile_pool(name="psum_o", bufs=3, space="PSUM") as psum_o,
        ):
            # ---- input loads, chunked + dual-queue so PE starts early ----
            xsb = persist.tile([128, NT * 256], bf16, name="xsb", tag="xsb")
            band = persist.tile([128, NT, 3, TW], bf16, name="band", tag="band")
            wsb = persist.tile([128, 12 * 128], bf16, name="wsb", tag="wsb")
            bsb = small.tile([128, 2], fp32, name="bsb", tag="bsb")

            def load_chunk(j0, j1):
                nc.sync.dma_start(
                    xsb[:, 256 * j0 : 256 * j1], xtp_d[:, 256 * j0 : 256 * j1]
                )
                nc.scalar.dma_start(
                    band[:, j0:j1, :, :],
                    band_d[:, 378 * j0 : 378 * j1].rearrange(
                        "p (j k t) -> p j k t", k=3, t=TW
                    ),
                )

            load_chunk(*lg[0])
            nc.scalar.dma_start(wsb[:, :], wt_d[:, :])
            nc.sync.dma_start(bsb[:, :], bias_d[:, :])
            for (j0, j1) in lg[1:]:
                load_chunk(j0, j1)

            # interp2[ch][p, k, 126 j + t]
            interp2 = [
                ipool.tile([128, 3, NT * TW], bf16, name=f"it{ch}", tag=f"it{ch}")
                for ch in range(2)
            ]

            def interp_tile(j):
                for ch in range(2):
                    pi = psum_i.tile([128, 3 * TW], fp32, name="pi", tag="pi")
                    nc.tensor.matmul(
                        pi[:, :],
                        xsb[:, 256 * j + 128 * ch : 256 * j + 128 * (ch + 1)],
                        band[:, j, :, :],
                        start=True,
                        stop=True,
                    )
                    dst = AP(interp2[ch].tensor, j * TW,
                             [(3 * NT * TW, 128), (NT * TW, 3), (1, TW)])
                    src = pi[:, :].rearrange("p (k t) -> p k t", k=3)
                    if (j * 2 + ch) % 2 == 0:
                        nc.scalar.copy(dst, src)
                    else:
                        nc.vector.tensor_scalar(dst, src, 0.0, None, Op.add)

            def main_group(chunks):
                for oh in range(2):
                    pos, ws = [], []
                    for c8 in chunks:
                        n0 = MW * c8
                        ws.append(min(MW, T - n0))
                        pos.append(
                            psum_o.tile([128, MW], fp32, name="po", tag="po")
                        )
                    for bi, (k, ch) in enumerate(
                        [(k, ch) for k in range(K) for ch in range(2)]
                    ):
                        idx = (k * 2 + ch) * 2 + oh
                        for ci, c8 in enumerate(chunks):
                            n0 = MW * c8
                            w = ws[ci]
                            nc.tensor.matmul(
                                pos[ci][:, :w],
                                wsb[:, 128 * idx : 128 * (idx + 1)],
                                interp2[ch][:, k, n0 : n0 + w],
                                start=(bi == 0),
                                stop=(bi == 5),
                            )
                    for ci, c8 in enumerate(chunks):
                        n0 = MW * c8
                        w = ws[ci]
                        ost = work.tile([128, MW], fp32, name="ost", tag="ost")
                        if ci % 2 == 1:
                            nc.vector.tensor_scalar(
                                ost[:, :w], pos[ci][:, :w],
                                bsb[:, oh : oh + 1], None, Op.add,
                            )
                        else:
                            nc.scalar.activation(
                                ost[:, :w], pos[ci][:, :w],
                                mybir.ActivationFunctionType.Identity,
                                bias=bsb[:, oh : oh + 1], scale=1.0,
                            )
                        nc.sync.dma_start(
                            out_d[128 * oh : 128 * (oh + 1), n0 : n0 + w],
                            ost[:, :w],
                        )

            # interleave: emit each main group right after the interp tiles
            # covering it, so PE fills copy-bound gaps with main matmuls
            next_group = 0
            for j in range(NT):
                interp_tile(j)
                if next_group < len(cgs):
                    last_chunk = cgs[next_group][-1]
                    need_j = min(
                        NT - 1, (MW * (last_chunk + 1) + TW - 1) // TW - 1
                    )
                    if j >= need_j:
                        main_group(cgs[next_group])
                        next_group += 1
            while next_group < len(cgs):
                main_group(cgs[next_group])
                next_group += 1
    _split_excess_waits(nc)
    return nc


def _prep_inputs(x, offsets, weight, bias):
    import ml_dtypes

    bf = ml_dtypes.bfloat16
    # xtp[p, j, c] = x[c, 126 j + p]  (zero beyond L)
    jj, pp = np.meshgrid(np.arange(NT), np.arange(128), indexing="ij")
    tt = TW * jj + pp  # [NT, 128]
    valid = tt < L
    tt_c = np.clip(tt, 0, L - 1)
    xtp_all = np.zeros((B, 128, NT, 256), np.float32)
    for b in range(B):
        g = x[b][:, tt_c]  # [256, NT, 128]
        g = np.where(valid[None, :, :], g, 0.0)
        xtp_all[b] = g.transpose(2, 1, 0)  # [128, NT, 256]
    xtp = xtp_all.reshape(B, 128, NT * 256).astype(bf)

    # band[p, j, k, t] = A_{k, p-t}[126 j + t]  (tent weights on diagonals)
    # s_k[t] = clip(offsets[t, k] + k, 0, 2); A_kd[t] = max(0, 1 - |s_k - d|)
    kk = np.arange(K, dtype=np.float32)
    dd = np.arange(3, dtype=np.float32)
    band_all = np.zeros((B, 128, NT, K, TW), np.float32)
    t_idx = np.arange(TW)
    for b in range(B):
        s = np.clip(offsets[b, 0] + kk[None, :], 0.0, 2.0)  # [T, K]
        A = np.maximum(0.0, 1.0 - np.abs(s[:, :, None] - dd[None, None, :]))
        A_pad = np.zeros((NT * TW, K, 3), np.float32)
        A_pad[:T] = A  # zero beyond T: padded columns produce zero interp
        A_r = A_pad.reshape(NT, TW, K, 3)
        for d in range(3):
            # rows p = t + d, cols t  (both sides index-advance to [126,NT,K])
            band_all[b, t_idx + d, :, :, t_idx] = A_r[:, t_idx, :, d]
    band_h = band_all.reshape(B, 128, NT * K * TW).astype(bf)

    wtt = np.zeros((128, K, 2, 2, 128), np.float32)
    for k in range(K):
        for ch in range(2):
            for oh in range(2):
                wtt[:, k, ch, oh, :] = weight[
                    128 * oh : 128 * (oh + 1), 128 * ch : 128 * (ch + 1), k
                ].T
    wt = wtt.reshape(128, 12 * 128).astype(bf)

    bias2 = bias.reshape(2, 128).T.astype(np.float32).copy()

    maps = []
    for b in range(B):
        maps.append(
            {
                "xtp": np.ascontiguousarray(xtp[b]),
                "band": np.ascontiguousarray(band_h[b]),
                "wt": wt,
                "bias": bias2,
            }
        )
    return maps


def kernel(x, offsets, weight, bias):
    from concourse import bass_utils

    x = np.asarray(x, np.float32)
    offsets = np.asarray(offsets, np.float32)
    weight = np.asarray(weight, np.float32)
    bias = np.asarray(bias, np.float32)

    if "nc" not in _CACHE:
        _CACHE["nc"] = _build_bass()
    nc = _CACHE["nc"]
    in_maps = _prep_inputs(x, offsets, weight, bias)
    res = bass_utils.run_bass_kernel_spmd(nc, in_maps, core_ids=list(range(B)))
    out = np.stack([res.results[b]["out"] for b in range(B)], axis=0)
    return out.astype(np.float32)
